# revision 1
# baseline (speedup 1.0000x reference)
"""Trainium2 Bass kernel for nn_Critic (MLP value function + GAE).

Sharding: batch B=2048 split across 8 NeuronCores (256 each). MLP params
replicated. The time recurrence (reverse GAE scan) is independent per batch
element, so no cross-core communication.

Per-core layout strategy:
  - states [17*256, 2048] processed one time step t at a time (256 rows).
  - natural-layout rows are PE-transposed (float32r, 1.5 cyc/row) into
    feature-major tiles stT [128 feat, 256 batch] which feed the matmul
    moving operand; weights are the stationary operand in natural layout.
  - all matmuls in float32r: full PE rate (1 cycle/row at N>=256) vs 4
    cycles/row for plain float32.
  - ELU(z) = min(exp(z)-1, relu(z)): one ScalarE Exp (fused +bias from
    PSUM) + one VectorE relu (fused +bias) + one VectorE combine.
  - value head uses h3 as the *stationary* operand so value lands
    [batch, 1] in PSUM -> values accumulate into valT [128, 17] tiles
    with time along the free axis (stored time-reversed).
  - GAE: deltas/scan/ret computed with a handful of [128,16] VectorE ops;
    the reverse scan is a single tensor_tensor_scan (state = dl*state + delta)
    since host pre-reverses reward/cont and valT is written reversed.
"""

import sys

sys.path.insert(0, "/opt/trn_rl_repo")

import numpy as np

T, B, D, H = 16, 2048, 2048, 1024
NCORES = 8
BC = B // NCORES  # 256 batch per core
TP1 = T + 1
DISCOUNT, LAMBDA = 0.99, 0.95
P = 128
KD = D // P  # 16 k-tiles for layer 0
KH = H // P  # 8 k-tiles for layers 1,2,out
MH = H // P  # 8 m-tiles of hidden units

_NC_CACHE = None


def _build():
    import concourse.bacc as bacc
    import concourse.mybir as mybir
    from concourse.tile import TileContext
    from concourse.masks import make_identity

    F32 = mybir.dt.float32
    BF16 = mybir.dt.bfloat16
    ALU = mybir.AluOpType
    ACTF = mybir.ActivationFunctionType

    nc = bacc.Bacc(None, target_bir_lowering=False, debug=False)

    states_h = nc.declare_dram_parameter("states", [TP1 * BC, D], F32, isOutput=False)
    rew_h = nc.declare_dram_parameter("rew_rev", [BC, T], F32, isOutput=False)
    cont_h = nc.declare_dram_parameter("cont_rev", [BC, TP1], F32, isOutput=False)
    w0hi_h = nc.declare_dram_parameter("W0hi", [D, H], BF16, isOutput=False)
    w0lo_h = nc.declare_dram_parameter("W0lo", [D, H], BF16, isOutput=False)
    b0_h = nc.declare_dram_parameter("b0", [H, 1], F32, isOutput=False)
    w1hi_h = nc.declare_dram_parameter("W1hi", [H, H], BF16, isOutput=False)
    w1lo_h = nc.declare_dram_parameter("W1lo", [H, H], BF16, isOutput=False)
    b1_h = nc.declare_dram_parameter("b1", [H, 1], F32, isOutput=False)
    w2hi_h = nc.declare_dram_parameter("W2hi", [H, H], BF16, isOutput=False)
    w2lo_h = nc.declare_dram_parameter("W2lo", [H, H], BF16, isOutput=False)
    b2_h = nc.declare_dram_parameter("b2", [H, 1], F32, isOutput=False)
    wo_h = nc.declare_dram_parameter("Wo", [H, 1], F32, isOutput=False)
    bo_h = nc.declare_dram_parameter("bo", [1, 1], F32, isOutput=False)
    ret_h = nc.declare_dram_parameter("ret_bt", [BC, T], F32, isOutput=True)
    val_h = nc.declare_dram_parameter("val_bt", [BC, T], F32, isOutput=True)

    with TileContext(nc) as tc:
        with (
            tc.tile_pool(name="wpool", bufs=1) as wpool,
            tc.tile_pool(name="spool", bufs=2) as spool,
            tc.tile_pool(name="stpool", bufs=3) as stpool,
            tc.tile_pool(name="hpool", bufs=1) as hpool,
            tc.tile_pool(name="tmp", bufs=3) as tmppool,
            tc.tile_pool(name="gae", bufs=1) as gaepool,
            tc.tile_pool(name="psA", bufs=4, space="PSUM") as psApool,
            tc.tile_pool(name="psT", bufs=2, space="PSUM") as psTpool,
            tc.tile_pool(name="psV", bufs=2, space="PSUM") as psVpool,
        ):
            # ---- persistent weights / constants ----
            def load_weight(dram_h, name, nk):
                tiles = []
                for k in range(nk):
                    wt = wpool.tile([P, H], BF16, name=f"{name}{k}", tag=f"{name}{k}")
                    nc.sync.dma_start(out=wt[:], in_=dram_h[k * P : (k + 1) * P, :])
                    tiles.append(wt)
                return tiles

            w0hi = load_weight(w0hi_h, "w0hi", KD)
            w0lo = load_weight(w0lo_h, "w0lo", KD)
            w1hi = load_weight(w1hi_h, "w1hi", KH)
            w1lo = load_weight(w1lo_h, "w1lo", KH)
            w2hi = load_weight(w2hi_h, "w2hi", KH)
            w2lo = load_weight(w2lo_h, "w2lo", KH)
            wosb = wpool.tile([P, KH], F32, name="wosb", tag="wosb")
            for k in range(KH):
                nc.sync.dma_start(out=wosb[:, k : k + 1], in_=wo_h[k * P : (k + 1) * P, :])
            bsb = []
            for li, bh in enumerate((b0_h, b1_h, b2_h)):
                bt = wpool.tile([P, MH], F32, name=f"bsb{li}", tag=f"bsb{li}")
                for m in range(MH):
                    nc.sync.dma_start(out=bt[:, m : m + 1], in_=bh[m * P : (m + 1) * P, :])
                bsb.append(bt)
            bosb = wpool.tile([1, 1], F32, name="bosb", tag="bosb")
            nc.sync.dma_start(out=bosb[:], in_=bo_h[:])
            ones_sb = wpool.tile([1, P], F32, name="ones_sb", tag="ones_sb")
            nc.vector.memset(ones_sb[:], 1.0)
            ident = wpool.tile([P, P], F32, name="ident", tag="ident")
            make_identity(nc, ident[:])

            valT = []
            for blk in range(2):
                vt = gaepool.tile([P, TP1], F32, name=f"valT{blk}", tag=f"valT{blk}")
                valT.append(vt)

            # GAE inputs can load up-front; they are consumed at the end.
            contsb = []
            rewsb = []
            for blk in range(2):
                ct = gaepool.tile([P, TP1], F32, name=f"contsb{blk}", tag=f"contsb{blk}")
                nc.sync.dma_start(out=ct[:], in_=cont_h[blk * P : (blk + 1) * P, :])
                contsb.append(ct)
                rt = gaepool.tile([P, T], F32, name=f"rewsb{blk}", tag=f"rewsb{blk}")
                nc.sync.dma_start(out=rt[:], in_=rew_h[blk * P : (blk + 1) * P, :])
                rewsb.append(rt)

            # ---- per-time-step fused MLP ----
            for t in range(TP1):
                snat = []
                for blk in range(2):
                    st = spool.tile([P, D], F32, name=f"snat{blk}", tag="snat", bufs=3)
                    row0 = t * BC + blk * P
                    nc.sync.dma_start(out=st[:], in_=states_h[row0 : row0 + P, :])
                    snat.append(st)

                # layer 0: transpose all k-tiles (fp32 exact), split each PSUM
                # block into bf16 hi/lo, then m-outer 3-pass contiguous groups
                sthi_tiles = []
                stlo_tiles = []
                for k in range(KD):
                    sthi = stpool.tile([P, BC], BF16, name=f"sthi{k}", tag="sthi", bufs=20)
                    stlo = stpool.tile([P, BC], BF16, name=f"stlo{k}", tag="stlo", bufs=20)
                    for blk in range(2):
                        pt = psTpool.tile([P, P], F32, name="pt", tag="pt")
                        nc.tensor.transpose(
                            pt[:], snat[blk][:, k * P : (k + 1) * P], ident[:]
                        )
                        hs = slice(blk * P, (blk + 1) * P)
                        nc.scalar.copy(sthi[:, hs], pt[:])
                        nc.vector.tensor_sub(stlo[:, hs], pt[:], sthi[:, hs])
                    sthi_tiles.append(sthi)
                    stlo_tiles.append(stlo)
                psms0 = []
                for m in range(MH):
                    ms = slice(m * P, (m + 1) * P)
                    psm = psApool.tile([P, BC], F32, name="psm", tag="psm")
                    for k in range(KD):
                        for pi, (wt, xt) in enumerate(
                            (
                                (w0hi[k], sthi_tiles[k]),
                                (w0hi[k], stlo_tiles[k]),
                                (w0lo[k], sthi_tiles[k]),
                            )
                        ):
                            nc.tensor.matmul(
                                psm[:],
                                lhsT=wt[:, ms],
                                rhs=xt[:],
                                start=(k == 0 and pi == 0),
                                stop=(k == KD - 1 and pi == 2),
                                skip_group_check=True,
                            )
                    psms0.append(psm)

                h1hi = hpool.tile([P, MH * BC], BF16, name="h1hi", tag="h1hi")
                h1lo = hpool.tile([P, MH * BC], BF16, name="h1lo", tag="h1lo")
                for m in range(MH):
                    ps = psms0[m][:]
                    e = tmppool.tile([P, BC], F32, name="e", tag="e")
                    nc.scalar.activation(e[:], ps, ACTF.Exp, bias=bsb[0][:, m : m + 1])
                    rl = tmppool.tile([P, BC], F32, name="rl", tag="rl")
                    nc.vector.tensor_scalar(
                        rl[:], ps, bsb[0][:, m : m + 1], 0.0, ALU.add, ALU.max
                    )
                    h32 = tmppool.tile([P, BC], F32, name="h32", tag="h32")
                    nc.vector.scalar_tensor_tensor(
                        h32[:], e[:], 1.0, rl[:], ALU.subtract, ALU.min
                    )
                    hs = slice(m * BC, (m + 1) * BC)
                    nc.scalar.copy(h1hi[:, hs], h32[:])
                    nc.vector.tensor_sub(h1lo[:, hs], h32[:], h1hi[:, hs])

                # layers 1 and 2 (layer 2 output h3 stays fp32 for the head)
                hinhi, hinlo = h1hi, h1lo
                h3 = None
                for li, (whi, wlo, bias) in enumerate(
                    ((w1hi, w1lo, bsb[1]), (w2hi, w2lo, bsb[2]))
                ):
                    last = li == 1
                    if last:
                        h3 = hpool.tile([P, MH * BC], F32, name="h3", tag="h3")
                    else:
                        houthi = hpool.tile([P, MH * BC], BF16, name="h2hi", tag="h2hi")
                        houtlo = hpool.tile([P, MH * BC], BF16, name="h2lo", tag="h2lo")
                    for m in range(MH):
                        ms = slice(m * P, (m + 1) * P)
                        psm = psApool.tile([P, BC], F32, name="psm", tag="psm")
                        for k in range(KH):
                            ks = slice(k * BC, (k + 1) * BC)
                            for pi, (wt, xs) in enumerate(
                                (
                                    (whi[k], hinhi[:, ks]),
                                    (whi[k], hinlo[:, ks]),
                                    (wlo[k], hinhi[:, ks]),
                                )
                            ):
                                nc.tensor.matmul(
                                    psm[:],
                                    lhsT=wt[:, ms],
                                    rhs=xs,
                                    start=(k == 0 and pi == 0),
                                    stop=(k == KH - 1 and pi == 2),
                                    skip_group_check=True,
                                )
                        ps = psm[:]
                        e = tmppool.tile([P, BC], F32, name="e", tag="e")
                        nc.scalar.activation(e[:], ps, ACTF.Exp, bias=bias[:, m : m + 1])
                        rl = tmppool.tile([P, BC], F32, name="rl", tag="rl")
                        nc.vector.tensor_scalar(
                            rl[:], ps, bias[:, m : m + 1], 0.0, ALU.add, ALU.max
                        )
                        hs = slice(m * BC, (m + 1) * BC)
                        if last:
                            nc.vector.scalar_tensor_tensor(
                                h3[:, hs], e[:], 1.0, rl[:], ALU.subtract, ALU.min
                            )
                        else:
                            h32 = tmppool.tile([P, BC], F32, name="h32", tag="h32")
                            nc.vector.scalar_tensor_tensor(
                                h32[:], e[:], 1.0, rl[:], ALU.subtract, ALU.min
                            )
                            nc.scalar.copy(houthi[:, hs], h32[:])
                            nc.vector.tensor_sub(houtlo[:, hs], h32[:], houthi[:, hs])
                    if not last:
                        hinhi, hinlo = houthi, houtlo

                # value head: h3 stationary, Wo moving -> value [batch, 1]
                for blk in range(2):
                    pv = psVpool.tile([P, 1], F32, name="pv", tag="pv")
                    for k in range(KH):
                        nc.tensor.matmul(
                            pv[:],
                            lhsT=h3[:, k * BC + blk * P : k * BC + blk * P + P],
                            rhs=wosb[:, k : k + 1],
                            start=(k == 0),
                            stop=False,
                            skip_group_check=True,
                        )
                    nc.tensor.matmul(
                        pv[:],
                        lhsT=ones_sb[:],
                        rhs=bosb[:],
                        start=False,
                        stop=True,
                        skip_group_check=True,
                    )
                    # store time-REVERSED: column 16-t
                    nc.scalar.copy(valT[blk][:, TP1 - 1 - t : TP1 - t], pv[:])

            # ---- GAE (all [128, 16/17] VectorE ops; time axis pre-reversed) ----
            for blk in range(2):
                disc = gaepool.tile([P, T], F32, name=f"disc{blk}", tag=f"disc{blk}")
                nc.vector.tensor_scalar_mul(disc[:], contsb[blk][:, 0:T], DISCOUNT)
                dtt = gaepool.tile([P, T], F32, name=f"dtt{blk}", tag=f"dtt{blk}")
                nc.vector.tensor_mul(dtt[:], disc[:], valT[blk][:, 0:T])
                nc.vector.tensor_add(dtt[:], dtt[:], rewsb[blk][:])
                nc.vector.tensor_sub(dtt[:], dtt[:], valT[blk][:, 1 : TP1])
                dl = gaepool.tile([P, T], F32, name=f"dl{blk}", tag=f"dl{blk}")
                nc.vector.tensor_scalar_mul(dl[:], disc[:], LAMBDA)
                adv = gaepool.tile([P, T], F32, name=f"adv{blk}", tag=f"adv{blk}")
                nc.vector.tensor_tensor_scan(
                    adv[:], dl[:], dtt[:], 0.0, ALU.mult, ALU.add
                )
                ret = gaepool.tile([P, T], F32, name=f"ret{blk}", tag=f"ret{blk}")
                nc.vector.tensor_add(ret[:], adv[:], valT[blk][:, 1 : TP1])
                nc.sync.dma_start(out=ret_h[blk * P : (blk + 1) * P, :], in_=ret[:])
                nc.sync.dma_start(
                    out=val_h[blk * P : (blk + 1) * P, :], in_=valT[blk][:, 1 : TP1]
                )

    nc.compile()
    return nc


def _get_nc():
    global _NC_CACHE
    if _NC_CACHE is None:
        _NC_CACHE = _build()
    return _NC_CACHE


def _make_in_maps(inputs):
    states = np.asarray(inputs["states"], dtype=np.float32)
    reward = np.asarray(inputs["reward"], dtype=np.float32)
    cont = np.asarray(inputs["cont"], dtype=np.float32)
    import ml_dtypes

    def split_bf16(w):
        w = np.ascontiguousarray(np.asarray(w, dtype=np.float32))
        hi = w.astype(ml_dtypes.bfloat16)
        lo = (w - hi.astype(np.float32)).astype(ml_dtypes.bfloat16)
        return np.ascontiguousarray(hi), np.ascontiguousarray(lo)

    W0hi, W0lo = split_bf16(inputs["W0"])
    W1hi, W1lo = split_bf16(inputs["W1"])
    W2hi, W2lo = split_bf16(inputs["W2"])
    Wo = np.ascontiguousarray(np.asarray(inputs["Wo"], dtype=np.float32).reshape(H, 1))
    b0 = np.ascontiguousarray(np.asarray(inputs["b0"], dtype=np.float32).reshape(H, 1))
    b1 = np.ascontiguousarray(np.asarray(inputs["b1"], dtype=np.float32).reshape(H, 1))
    b2 = np.ascontiguousarray(np.asarray(inputs["b2"], dtype=np.float32).reshape(H, 1))
    bo = np.ascontiguousarray(np.asarray(inputs["bo"], dtype=np.float32).reshape(1, 1))

    in_maps = []
    for c in range(NCORES):
        sl = slice(c * BC, (c + 1) * BC)
        in_maps.append(
            {
                "states": np.ascontiguousarray(states[:, sl, :]).reshape(TP1 * BC, D),
                "rew_rev": np.ascontiguousarray(reward[::-1, sl].T),
                "cont_rev": np.ascontiguousarray(cont[::-1, sl].T),
                "W0hi": W0hi,
                "W0lo": W0lo,
                "b0": b0,
                "W1hi": W1hi,
                "W1lo": W1lo,
                "b1": b1,
                "W2hi": W2hi,
                "W2lo": W2lo,
                "b2": b2,
                "Wo": Wo,
                "bo": bo,
            }
        )
    return in_maps


def _run(inputs, trace=False):
    from concourse.bass_utils import run_bass_kernel_spmd

    nc = _get_nc()
    in_maps = _make_in_maps(inputs)
    bkr = run_bass_kernel_spmd(nc, in_maps, list(range(NCORES)), trace=trace)
    ret = np.empty((T, B), np.float32)
    val = np.empty((T, B), np.float32)
    for c in range(NCORES):
        sl = slice(c * BC, (c + 1) * BC)
        ret[:, sl] = bkr.results[c]["ret_bt"].T[::-1]
        val[:, sl] = bkr.results[c]["val_bt"].T[::-1]
    return (ret, val), bkr


def kernel(**inputs):
    out, _ = _run(inputs, trace=False)
    return out

